# revision 1
# baseline (speedup 1.0000x reference)
"""Causal multi-head self-attention on 8 trn2 NeuronCores.

Sharding: 8 cores = batch(4) x head-group(2).  Each core computes attention
for 6 of the 12 heads of one batch element, plus its partial output
projection; the host sums the two partials per batch element.

Per-core kernel (Bass/Tile, all matmuls in float32r = full-rate fp32):
  v = x @ wv.T first ([S, 384] natural, + ones col per head), then per
  head-pair ht: project qT/kT m-tile ht, immediately followed by attention
  for heads 2ht, 2ht+1 so ScalarE exp work overlaps remaining projections:
    scoresT[k,q] = kT_h slices.T @ qT_h   (k on partitions, PSUM)
    pT = exp(scoresT)  (ScalarE reads PSUM) -> causal zero via
    gpsimd.affine_select (diagonal prefix only) -> oT[65,512] += v_ext.T @ pT
    (ones column of v_ext yields the softmax denominator as row 64 free)
    normalize: attnT = oT[0:64] * (1/oT[64]) broadcast, written over qT
  finally out[S,768] partial = attnT.T @ woT
"""

import numpy as np

import concourse.bass as bass
import concourse.bacc as bacc
import concourse.mybir as mybir
import concourse.tile as tile
from concourse.bass_utils import run_bass_kernel_spmd

F32 = mybir.dt.float32
F32R = mybir.dt.float32r

B, S, D = 4, 2048, 768
H = 12          # total heads
DK = 64         # head dim
HPC = 6         # heads per core
GC = HPC * DK   # 384 cols per head-group
P = 128
KT = D // P     # 6 k-tiles over d_model
MT = GC // P    # 3 tiles over the 384 group cols
NQ = S // 512   # 4 q chunks of 512
SBLK = S // P   # 16 sequence blocks of 128


def _attention_head(nc, ps_sc, ps_o, att_w, att_n, qa_sb, kT_sb, v_sb, h, no_mask=False):
    hp = 64 * (h % 2)
    ht = h // 2
    for j in range(NQ):
        nblk = 4 * (j + 1)  # causal: k blocks 0..nblk-1
        po = ps_o.tile([DK + 1, 512], F32, name="po", tag="po")
        for g in range(nblk // 2):
            ps = ps_sc.tile([P, 2, 512], F32, name="ps", tag="ps")
            for i in range(2):
                b = 2 * g + i
                nc.tensor.matmul(
                    ps[:, i, :],
                    kT_sb[hp : hp + DK, ht, b * P : (b + 1) * P],
                    qa_sb[hp : hp + DK, ht, j * 512 : (j + 1) * 512],
                    start=True,
                    stop=True,
                )
            pt = att_w.tile([P, 2, 512], F32R, name="pt", tag="pt")
            nc.scalar.activation(pt[:], ps[:], mybir.ActivationFunctionType.Exp)
            for i in range(2):
                b = 2 * g + i
                off = 512 * j - 128 * b
                if not no_mask and 0 <= 128 * b - 512 * j < 512:
                    # diagonal block: zero pT where k > q; only columns
                    # < 128*(bi+1) can be masked, restrict to that prefix
                    bi = 128 * b - 512 * j
                    w = bi + 128
                    nc.gpsimd.affine_select(
                        out=pt[:, i, 0:w],
                        in_=pt[:, i, 0:w],
                        compare_op=mybir.AluOpType.is_ge,
                        fill=0.0,
                        base=off,
                        pattern=[[1, w]],
                        channel_multiplier=-1,
                    )
            for i in range(2):
                b = 2 * g + i
                nc.tensor.matmul(
                    po[:],
                    v_sb[:, b, h, :],
                    pt[:, i, :],
                    start=(b == 0),
                    stop=(b == nblk - 1),
                )
        # normalize: attnT = po[0:64] / po[64], written into qT's storage.
        # NB: partition_broadcast reads PHYSICAL partition 0 on HW (ignores
        # the AP base), so land the reciprocal at base 0 first (DVE handles
        # the cross-partition-base shift).
        rec = att_n.tile([1, 512], F32, name="rec", tag="rec")
        nc.vector.reciprocal(rec[:], po[DK : DK + 1, :])
        recb = att_n.tile([DK, 512], F32, name="recb", tag="recb")
        nc.gpsimd.partition_broadcast(recb[:], rec[:])
        nc.vector.tensor_mul(
            qa_sb[hp : hp + DK, ht, j * 512 : (j + 1) * 512],
            po[0:DK, :],
            recb[:],
        )


def _emit(nc, tc, d, r, no_mask=False):
    """Emit one full forward pass. d = dict of DRAM APs, r = rep index."""
    with tc.tile_pool(name=f"persist{r}", bufs=1) as persist:
        qa_sb = persist.tile([P, MT, S], F32R, name="qa_sb")   # qT, then attnT
        kT_sb = persist.tile([P, MT, S], F32R, name="kT_sb")
        # v natural + ones column per head: [p, sblk, head, 65]
        v_sb = persist.tile([P, SBLK, HPC, DK + 1], F32R, name="v_sb")
        wo_sb = persist.tile([P, MT, D], F32R, name="wo_sb")

        # memset can't produce f32r; write ones via f32 memset + DVE copy
        ones_f32 = persist.tile([P, SBLK * HPC], F32, name="ones_f32")
        nc.vector.memset(ones_f32[:], 1.0)
        nc.vector.tensor_copy(
            v_sb[:, :, :, DK], ones_f32.rearrange("p (t h) -> p t h", t=SBLK)
        )
        nc.sync.dma_start(wo_sb[:], d["woT"].rearrange("(t p) n -> p t n", p=P))

        with tc.tile_pool(name=f"xw{r}", bufs=1) as xw:
            xT_sb = xw.tile([P, KT, S], F32R, name="xT_sb")
            wq_sb = xw.tile([P, KT, GC], F32R, name="wq_sb")
            wk_sb = xw.tile([P, KT, GC], F32R, name="wk_sb")
            wv_sb = xw.tile([P, KT, GC], F32R, name="wv_sb")

            nc.sync.dma_start(wv_sb[:], d["wvT"].rearrange("(t p) n -> p t n", p=P))
            xT_r = d["xT"].rearrange("(t p) s -> p t s", p=P)
            for k in range(KT):
                nc.sync.dma_start(xT_sb[:, k, :], xT_r[:, k, :])
            nc.sync.dma_start(wq_sb[:], d["wqT"].rearrange("(t p) n -> p t n", p=P))
            nc.sync.dma_start(wk_sb[:], d["wkT"].rearrange("(t p) n -> p t n", p=P))

            # ---- phase 1: projections ----
            with tc.tile_pool(name=f"ps1{r}", bufs=4, space="PSUM") as ps1:
                for w_sb, dst in ((wq_sb, qa_sb), (wk_sb, kT_sb)):
                    for m in range(MT):
                        for n in range(NQ):
                            pq = ps1.tile([P, 512], F32, name="pq", tag="pq")
                            for k in range(KT):
                                nc.tensor.matmul(
                                    pq[:],
                                    w_sb[:, k, m * P : (m + 1) * P],
                                    xT_sb[:, k, n * 512 : (n + 1) * 512],
                                    start=(k == 0),
                                    stop=(k == KT - 1),
                                )
                            nc.vector.tensor_copy(
                                dst[:, m, n * 512 : (n + 1) * 512], pq[:]
                            )
                for t in range(SBLK):
                    pv = ps1.tile([P, GC], F32, name="pv", tag="pq")
                    for k in range(KT):
                        nc.tensor.matmul(
                            pv[:],
                            xT_sb[:, k, t * P : (t + 1) * P],
                            wv_sb[:, k, :],
                            start=(k == 0),
                            stop=(k == KT - 1),
                        )
                    nc.vector.tensor_copy(
                        v_sb[:, t, :, 0:DK], pv.rearrange("p (h d) -> p h d", h=HPC)
                    )

        # ---- phase 2: attention ----
        with (
            tc.tile_pool(name=f"ps_sc{r}", bufs=2, space="PSUM") as ps_sc,
            tc.tile_pool(name=f"ps_o{r}", bufs=4, space="PSUM") as ps_o,
            tc.tile_pool(name=f"att_w{r}", bufs=3) as att_w,
            tc.tile_pool(name=f"att_n{r}", bufs=2) as att_n,
        ):
            for h in range(HPC):
                _attention_head(
                    nc, ps_sc, ps_o, att_w, att_n, qa_sb, kT_sb, v_sb, h,
                    no_mask=no_mask,
                )

        # ---- phase 3: output projection ----
        with (
            tc.tile_pool(name=f"ps3{r}", bufs=4, space="PSUM") as ps3,
            tc.tile_pool(name=f"out_w{r}", bufs=3) as out_w,
        ):
            for t in range(SBLK):
                ot = out_w.tile([P, D], F32, name="ot", tag="ot")
                for n in range(2):
                    po3 = ps3.tile([P, 384], F32, name="po3", tag="po3")
                    for k in range(MT):
                        nc.tensor.matmul(
                            po3[:],
                            qa_sb[:, k, t * P : (t + 1) * P],
                            wo_sb[:, k, n * 384 : (n + 1) * 384],
                            start=(k == 0),
                            stop=(k == MT - 1),
                        )
                    nc.vector.tensor_copy(ot[:, n * 384 : (n + 1) * 384], po3[:])
                nc.sync.dma_start(d["out"][t * P : (t + 1) * P, :], ot[:])


def build_nc(debug_taps=False, reps=1, no_mask=False, interleave=False):
    nc = bacc.Bacc("TRN2", target_bir_lowering=False, debug=False)

    d = {
        "xT": nc.dram_tensor("xT", [D, S], F32R, kind="ExternalInput").ap(),
        "wqT": nc.dram_tensor("wqT", [D, GC], F32R, kind="ExternalInput").ap(),
        "wkT": nc.dram_tensor("wkT", [D, GC], F32R, kind="ExternalInput").ap(),
        "wvT": nc.dram_tensor("wvT", [D, GC], F32R, kind="ExternalInput").ap(),
        "woT": nc.dram_tensor("woT", [GC, D], F32R, kind="ExternalInput").ap(),
        "out": nc.dram_tensor("out", [S, D], F32, kind="ExternalOutput").ap(),
    }

    with tile.TileContext(nc) as tc:
        for r in range(reps):
            if interleave:
                _emit_interleaved(nc, tc, d, r, no_mask=no_mask)
            else:
                _emit(nc, tc, d, r, no_mask=no_mask)

    nc.compile()
    return nc


_NC = None


def _get_nc():
    global _NC
    if _NC is None:
        _NC = build_nc()
    return _NC


def make_in_maps(x, wq, wk, wv, wo):
    x = np.asarray(x, np.float32)
    wq = np.asarray(wq, np.float32)
    wk = np.asarray(wk, np.float32)
    wv = np.asarray(wv, np.float32)
    wo = np.asarray(wo, np.float32)
    scale = 1.0 / np.sqrt(np.float32(DK))
    in_maps = []
    for c in range(8):
        b, g = divmod(c, 2)
        sl = slice(GC * g, GC * (g + 1))
        in_maps.append(
            {
                "xT": np.ascontiguousarray(x[b].T),
                "wqT": np.ascontiguousarray((wq[sl, :] * scale).T),
                "wkT": np.ascontiguousarray(wk[sl, :].T),
                "wvT": np.ascontiguousarray(wv[sl, :].T),
                "woT": np.ascontiguousarray(wo[:, sl].T),
            }
        )
    return in_maps


def combine(results):
    outs = [np.asarray(r["out"], np.float32) for r in results]
    return np.stack([outs[2 * b] + outs[2 * b + 1] for b in range(B)])


def kernel(x, wq, wk, wv, wo, _trace=False):
    nc = _get_nc()
    res = run_bass_kernel_spmd(
        nc, make_in_maps(x, wq, wk, wv, wo), core_ids=list(range(8)), trace=_trace
    )
    out = combine(res.results)
    kernel.last_result = res
    return out


def _emit_interleaved(nc, tc, d, r, no_mask=False):
    """v projection first, then per head-pair: qT/kT projection immediately
    followed by that pair's attention, so exp work starts early."""
    with (
        tc.tile_pool(name=f"ipersist{r}", bufs=1) as persist,
        tc.tile_pool(name=f"ips1{r}", bufs=2, space="PSUM") as ps1,
        tc.tile_pool(name=f"ips_sc{r}", bufs=2, space="PSUM") as ps_sc,
        tc.tile_pool(name=f"ips_o{r}", bufs=2, space="PSUM") as ps_o,
        tc.tile_pool(name=f"iatt_w{r}", bufs=3) as att_w,
        tc.tile_pool(name=f"iatt_n{r}", bufs=2) as att_n,
        tc.tile_pool(name=f"iout_w{r}", bufs=3) as out_w,
    ):
        qa_sb = persist.tile([P, MT, S], F32R, name="qa_sb")
        kT_sb = persist.tile([P, MT, S], F32R, name="kT_sb")
        v_sb = persist.tile([P, SBLK, HPC, DK + 1], F32R, name="v_sb")
        wo_sb = persist.tile([P, MT, D], F32R, name="wo_sb")
        xT_sb = persist.tile([P, KT, S], F32R, name="xT_sb")
        wq_sb = persist.tile([P, KT, GC], F32R, name="wq_sb")
        wk_sb = persist.tile([P, KT, GC], F32R, name="wk_sb")
        wv_sb = persist.tile([P, KT, GC], F32R, name="wv_sb")

        ones_f32 = persist.tile([P, SBLK * HPC], F32, name="ones_f32")
        nc.vector.memset(ones_f32[:], 1.0)
        nc.vector.tensor_copy(
            v_sb[:, :, :, DK], ones_f32.rearrange("p (t h) -> p t h", t=SBLK)
        )

        nc.sync.dma_start(wv_sb[:], d["wvT"].rearrange("(t p) n -> p t n", p=P))
        xT_r = d["xT"].rearrange("(t p) s -> p t s", p=P)
        for k in range(KT):
            nc.sync.dma_start(xT_sb[:, k, :], xT_r[:, k, :])
        nc.sync.dma_start(wq_sb[:], d["wqT"].rearrange("(t p) n -> p t n", p=P))
        nc.sync.dma_start(wk_sb[:], d["wkT"].rearrange("(t p) n -> p t n", p=P))
        nc.sync.dma_start(wo_sb[:], d["woT"].rearrange("(t p) n -> p t n", p=P))

        for t in range(SBLK):
            pv = ps1.tile([P, GC], F32, name="pv", tag="pq")
            for k in range(KT):
                nc.tensor.matmul(
                    pv[:], xT_sb[:, k, t * P : (t + 1) * P], wv_sb[:, k, :],
                    start=(k == 0), stop=(k == KT - 1),
                )
            nc.vector.tensor_copy(
                v_sb[:, t, :, 0:DK], pv.rearrange("p (h d) -> p h d", h=HPC)
            )

        for ht in range(MT):
            for w_sb, dst in ((wq_sb, qa_sb), (wk_sb, kT_sb)):
                for n in range(NQ):
                    pq = ps1.tile([P, 512], F32, name="pq", tag="pq")
                    for k in range(KT):
                        nc.tensor.matmul(
                            pq[:],
                            w_sb[:, k, ht * P : (ht + 1) * P],
                            xT_sb[:, k, n * 512 : (n + 1) * 512],
                            start=(k == 0), stop=(k == KT - 1),
                        )
                    nc.vector.tensor_copy(dst[:, ht, n * 512 : (n + 1) * 512], pq[:])
            _attention_head(nc, ps_sc, ps_o, att_w, att_n, qa_sb, kT_sb, v_sb,
                            2 * ht, no_mask=no_mask)
            _attention_head(nc, ps_sc, ps_o, att_w, att_n, qa_sb, kT_sb, v_sb,
                            2 * ht + 1, no_mask=no_mask)

        for t in range(SBLK):
            ot = out_w.tile([P, D], F32, name="ot", tag="ot")
            for n in range(2):
                po3 = ps1.tile([P, 384], F32, name="po3", tag="pq")
                for k in range(MT):
                    nc.tensor.matmul(
                        po3[:],
                        qa_sb[:, k, t * P : (t + 1) * P],
                        wo_sb[:, k, n * 384 : (n + 1) * 384],
                        start=(k == 0), stop=(k == MT - 1),
                    )
                nc.vector.tensor_copy(ot[:, n * 384 : (n + 1) * 384], po3[:])
            nc.sync.dma_start(d["out"][t * P : (t + 1) * P, :], ot[:])

